# revision 23
# baseline (speedup 1.0000x reference)
"""Expert-parallel MoE MLP kernel for Trainium2 (8 NeuronCores).

Problem: out[b,e,n,d] = gelu(x[b,e] @ w1[e] + b1[e]) @ w2[e] + b2[e]
Shapes: x [2,8,1024,1024] f32, w1 [8,1024,4096], b1 [8,4096],
        w2 [8,4096,1024], b2 [8,1024].

Sharding: expert e -> core e. Each core runs a 2048-token MLP:
  [2048,1024] @ [1024,4096] -> gelu -> @ [4096,1024] -> [2048,1024]

Device-side layout: activations live transposed ([feature, token]) so the
contraction dim is always the SBUF partition dim:
  phase 1: psum[h_tile, t] += w1[d_tile, h_tile].T @ xT[d_tile, t]
  phase 2: psum[d_tile, t] += w2[h_tile, d_tile].T @ hT[h_tile, t]
Host transposes x on the way in and out on the way back (part of
shard/unshard), so the device does zero transposes.

All matmul inputs are bf16 (fp32 PSUM accumulation); GELU (tanh approx,
matching jax.nn.gelu default) fused with the b1 add on ScalarE.

Notes from measurement (fp8 experiments abandoned): fp8 e4m3 DoubleRow /
DoubleRowSwInterleave matmuls run the PE stream at 2x, but on real TRN2 the
256-row stationary loads serialize against the 256-cycle streams (and plain
DoubleRow intermittently wedges the PE), so the 1.5x instruction count of
the split-precision scheme nets out slower than plain bf16 (756-883us vs
~508us per pass). bf16 keeps ldweights (128 rows) hidden under 512-cycle
streams and is stable.

w1 arrives tile-contiguous ([nh, P, kd*P], host-packed) so each weight-tile
DMA moves 2KB contiguous runs per partition: sub-512B descriptor runs pay a
2x DMA bandwidth penalty, which matters for how fast the first block's
weights land (startup is weight-DMA limited). x streams per 512-token block
through the gpsimd (SWDGE) queue so it never queues behind the 16 MiB of
weight DMAs on the sync queue.
"""

import sys

for _p in ("/opt/trn_rl_repo",):
    if _p not in sys.path:
        sys.path.insert(0, _p)

import numpy as np
import ml_dtypes

from contextlib import ExitStack

import concourse.tile as tile
from concourse import bacc, mybir
from concourse.bass_utils import run_bass_kernel_spmd

BF16 = mybir.dt.bfloat16
F32 = mybir.dt.float32

# Full-problem constants (hardcoded per harness contract).
B, E, N, D, H = 2, 8, 1024, 1024, 4096
T = B * N          # tokens per expert/core
TBLK = 512         # tokens per block (= one PSUM bank of fp32)
P = 128


def build_nc(t=T, d=D, h=H, tblk=TBLK, act=None, repeats=1,
             ps_bufs=4, act_mode="gelu", x_mode="stream"):
    """Build the per-core Bass program. All cores run this same program on
    different data (SPMD). repeats>1 re-runs the token-block loop (weights
    stay resident) — used only for steady-state timing measurements."""
    if act is None:
        act = mybir.ActivationFunctionType.Gelu_apprx_tanh
    kd = d // P        # contraction tiles for phase 1
    nh = h // P        # h tiles (phase-1 outputs / phase-2 contraction)
    nd = d // P        # d tiles (phase-2 outputs)
    nblk = t // tblk

    nc = bacc.Bacc("TRN2", target_bir_lowering=False)

    xt_hbm = nc.dram_tensor("xt", [d, t], BF16, kind="ExternalInput").ap()
    # w1 pre-packed tile-contiguous on host: [nh, P, kd, P]
    w1_hbm = nc.dram_tensor("w1", [nh, P, kd, P], BF16,
                            kind="ExternalInput").ap()
    w2_hbm = nc.dram_tensor("w2", [h, d], BF16, kind="ExternalInput").ap()
    b1_hbm = nc.dram_tensor("b1", [nh, P], F32, kind="ExternalInput").ap()
    b2_hbm = nc.dram_tensor("b2", [nd, P], F32, kind="ExternalInput").ap()
    # bf16 output: halves the output DMA drain; host upcasts to f32.
    # Adds ~1.1e-3 rounding on top of ~3.4e-3 matmul error — well inside
    # the 2e-2 gate.
    out_hbm = nc.dram_tensor("outT", [d, t], BF16, kind="ExternalOutput").ap()

    # [feature, x] views with the 128-partition dim innermost in features.
    xt_v = xt_hbm.rearrange("(kd p) t -> p kd t", p=P)
    w2_v = w2_hbm.rearrange("(kh p) d -> p kh d", p=P)

    with tile.TileContext(nc) as tc, ExitStack() as ctx:
        w1_pool = ctx.enter_context(tc.tile_pool(name="w1", bufs=nh))
        w2_pool = ctx.enter_context(tc.tile_pool(name="w2", bufs=nh))
        x_pool = ctx.enter_context(tc.tile_pool(name="x", bufs=2))
        h_pool = ctx.enter_context(tc.tile_pool(name="h", bufs=nh + 2))
        o_pool = ctx.enter_context(tc.tile_pool(name="o", bufs=4))
        c_pool = ctx.enter_context(tc.tile_pool(name="c", bufs=1))
        ps1 = ctx.enter_context(tc.tile_pool(name="ps1", bufs=ps_bufs, space="PSUM"))
        ps2 = ctx.enter_context(tc.tile_pool(name="ps2", bufs=ps_bufs, space="PSUM"))

        # Biases, resident.
        b1_sb = c_pool.tile([P, nh], F32)
        nc.sync.dma_start(out=b1_sb, in_=b1_hbm.rearrange("t p -> p t"))
        b2_sb = c_pool.tile([P, nd], F32)
        nc.sync.dma_start(out=b2_sb, in_=b2_hbm.rearrange("t p -> p t"))

        # Weights, resident in SBUF for the whole kernel. Chunked DMAs so
        # compute can start as soon as the first chunks land.
        w1_t = []
        for ih in range(nh):
            wt = w1_pool.tile([P, kd, P], BF16)
            nc.sync.dma_start(out=wt, in_=w1_hbm[ih])
            w1_t.append(wt)
        w2_t = []
        for ikh in range(nh):
            wt = w2_pool.tile([P, d], BF16)
            nc.sync.dma_start(out=wt, in_=w2_v[:, ikh, :])
            w2_t.append(wt)

        MM = nc.tensor.matmul

        xt_pre = {}
        if x_mode == "preload":
            for ib in range(nblk):
                xt_pre[ib] = c_pool.tile([P, kd, tblk], BF16,
                                         name=f"xp{ib}", tag=f"xp{ib}")
                nc.gpsimd.dma_start(
                    out=xt_pre[ib],
                    in_=xt_v[:, :, ib * tblk:(ib + 1) * tblk])
        for ib in [i % nblk for i in range(nblk * repeats)]:
            tsl = slice(ib * tblk, (ib + 1) * tblk)
            if x_mode == "preload":
                xt = xt_pre[ib]
            else:
                # gpsimd (SWDGE) queue: keeps x off the weight-DMA queue.
                xt = x_pool.tile([P, kd, tblk], BF16)
                nc.gpsimd.dma_start(out=xt, in_=xt_v[:, :, tsl])

            # phase 1: hT[h_tile] = gelu(w1.T @ xT + b1)
            ht = []
            for ih in range(nh):
                ps = ps1.tile([P, tblk], F32)
                for ik in range(kd):
                    MM(
                        ps, w1_t[ih][:, ik, :], xt[:, ik, :],
                        start=(ik == 0), stop=(ik == kd - 1),
                    )
                hs = h_pool.tile([P, tblk], BF16)
                if act_mode == "gelu":
                    nc.scalar.activation(hs, ps, act, bias=b1_sb[:, ih:ih + 1])
                else:
                    nc.vector.tensor_copy(hs, ps)
                ht.append(hs)

            # phase 2: outT[d_tile] = w2.T @ hT + b2
            for idt in range(nd):
                ps = ps2.tile([P, tblk], F32)
                for ikh in range(nh):
                    MM(
                        ps, w2_t[ikh][:, idt * P:(idt + 1) * P], ht[ikh],
                        start=(ikh == 0), stop=(ikh == nh - 1),
                    )
                ob = o_pool.tile([P, tblk], BF16)
                nc.vector.tensor_scalar_add(ob, ps, b2_sb[:, idt:idt + 1])
                nc.scalar.dma_start(
                    out=out_hbm[idt * P:(idt + 1) * P, tsl], in_=ob
                )

    nc.compile()
    return nc


_NC_CACHE = {}


def _get_nc():
    if "nc" not in _NC_CACHE:
        _NC_CACHE["nc"] = build_nc()
    return _NC_CACHE["nc"]


def _pack_w1(a, kd, nh):
    """[D, H] bf16 -> tile-contiguous [nh, P, kd, P]."""
    return np.ascontiguousarray(
        a.reshape(kd, P, nh, P).transpose(2, 1, 0, 3))


def make_in_map(xe, w1e, b1e, w2e, b2e):
    """Build the per-core input map from one expert's f32 slices.
    xe: [T, D]; w1e: [D, H]; b1e: [H]; w2e: [H, D]; b2e: [D]."""
    bf16 = ml_dtypes.bfloat16
    d, h = w1e.shape
    kd, nh = d // P, h // P
    xtb = np.ascontiguousarray(np.asarray(xe, np.float32).T).astype(bf16)
    return {
        "xt": xtb,
        "w1": _pack_w1(np.asarray(w1e, np.float32).astype(bf16), kd, nh),
        "w2": np.asarray(w2e, np.float32).astype(bf16),
        "b1": np.ascontiguousarray(
            np.asarray(b1e, np.float32).reshape(h // P, P)),
        "b2": np.ascontiguousarray(
            np.asarray(b2e, np.float32).reshape(d // P, P)),
    }


def kernel(x, w1, b1, w2, b2):
    nc = _get_nc()
    in_maps = []
    for e in range(E):
        xe = np.asarray(x[:, e], dtype=np.float32).reshape(T, D)
        in_maps.append(make_in_map(xe, w1[e], b1[e], w2[e], b2[e]))

    res = run_bass_kernel_spmd(nc, in_maps, core_ids=list(range(E)))

    out = np.empty((B, E, N, D), dtype=np.float32)
    for e in range(E):
        ot = np.asarray(res.results[e]["outT"]).astype(np.float32)  # [D, T]
        out[:, e] = ot.T.reshape(B, N, D)
    return out
